# revision 3
# baseline (speedup 1.0000x reference)
"""Multi-head self-attention (no causal mask) on 8 Trainium2 NeuronCores — v2.

Problem: B=2, S=2048, D=768, H=12 heads (head_dim 64), fp32 in/out.
Sharding: batch x head-group. Core c handles batch c//4 and heads
3*(c%4) .. 3*(c%4)+2. Each core computes a partial [2048, 768] output;
the host sums the 4 partials per batch and adds bo.

v2 design (vs v1): head-serial passes with PE tile-position pairing.
HW microbenchmarks showed serial matmuls cost ~265-405ns (LDWEIGHTS
cannot hide behind a matmul on the same PE tile quadrants) while
row/col-disjoint pairs run concurrently at ~85ns/MM. So every matmul
is emitted as a disjoint-quadrant pair:
  - scores: K=64 row pairs (c0 on PE rows 0-63, c1 on rows 64-127)
  - Q/K/V projections: M=64 col pairs (out partitions 0:64 / 64:128)
  - attn@V: M=64 col pairs; po rows 0:64 = c0, rows 64:128 = c1,
    accumulated over all 16 k-tiles in one PSUM bank. The softmax
    denominators get their own M=1 col-paired matmuls into pd.
  - Wo: attnT-A (rows 0:64) runs concurrently with attnT-B/C
    (rows 64:128) using two PSUM banks summed on DVE.
Concurrent same-element accumulation is ILLEGAL (hardware error) —
pairs always write disjoint PSUM elements; same-element chains stay
on conflicting quadrants so the PE serializes them.

The softmax scale 1/8 rides the exp activation's free affine.
ScalarE (exp, 96 x [128,1024]) is the intended bottleneck (~110us);
PE work is ~85us woven underneath it.

PSUM: 2x ps [128,1024] (4 banks) + po (1) + pd (1) + pw (2) = 8.
"""

import numpy as np

_CACHE = {}

S = 2048
D = 768
HLOC = 3          # heads per core
NKT = 6           # 768 / 128 d-tiles


def _emit(nc, tc, ctx, dram, loop_n=None, phase=5):
    import concourse.mybir as mybir

    f32 = mybir.dt.float32
    bf16 = mybir.dt.bfloat16
    add = mybir.AluOpType.add
    mult = mybir.AluOpType.mult
    Exp = mybir.ActivationFunctionType.Exp

    consts = ctx.enter_context(tc.tile_pool(name="consts", bufs=1))
    ppool = ctx.enter_context(tc.tile_pool(name="ppool", bufs=18))
    pps = ctx.enter_context(tc.tile_pool(name="pps", bufs=2, space="PSUM"))
    ppo = ctx.enter_context(tc.tile_pool(name="ppo", bufs=1, space="PSUM"))
    ppd = ctx.enter_context(tc.tile_pool(name="ppd", bufs=1, space="PSUM"))
    ppw = ctx.enter_context(tc.tile_pool(name="ppw", bufs=2, space="PSUM"))
    opool = ctx.enter_context(tc.tile_pool(name="opool", bufs=3))
    bpool = ctx.enter_context(tc.tile_pool(name="bpool", bufs=2))
    rpool = ctx.enter_context(tc.tile_pool(name="rpool", bufs=2))
    spool = ctx.enter_context(tc.tile_pool(name="spool", bufs=2))

    # ---- persistent SBUF tensors ----
    xt = consts.tile([128, NKT, S], bf16)          # x^T (DMA'd pre-transposed)
    qt = [consts.tile([128, 1024], bf16, name=f"qt{h}") for h in range(HLOC)]
    # qt[h] block cp (cols cp*512..): rows 0:64 = Q_h[q cp*1024+0:512] (c0),
    #                                rows 64:128 = Q_h[q cp*1024+512:1024] (c1)
    kt = [consts.tile([128, S], bf16, name=f"kt{h}") for h in range(HLOC)]
    # kt[h]: rows 0:64 = K_h^T (all k), rows 64:128 = duplicate
    v_sb = consts.tile([128, 16, HLOC, 64], bf16)  # V natural
    attnT = consts.tile([128, S], bf16)            # attn out^T heads A,B
    attnTc = consts.tile([64, S], bf16)            # head C

    w_q = consts.tile([128, NKT, 192], bf16)
    w_k = consts.tile([128, NKT, 192], bf16)
    w_v = consts.tile([128, NKT, 192], bf16)
    w_oab = consts.tile([128, D], bf16)
    w_oc = consts.tile([64, D], bf16)
    bq_d = consts.tile([128, HLOC], f32)           # per-head bias dup'd halves
    bk_d = consts.tile([128, HLOC], f32)
    bv_bc = consts.tile([128, HLOC * 64], f32)
    ones_c = consts.tile([128, 32], bf16)          # denominator lhsT (col 0)
    dmy = consts.tile([1, 8], f32)
    dmy2 = consts.tile([1, 8], f32)

    # ---- prologue: warm the exp table while weights stream in ----
    nc.vector.memset(dmy, 0.0)
    nc.scalar.activation(out=dmy2, in_=dmy, func=Exp)
    nc.vector.memset(ones_c, 0.0)
    nc.vector.memset(ones_c[:, 0:1], 1.0)

    # weight/bias DMAs ride the Activation queue (idle until first exp)
    nc.scalar.dma_start(out=w_q, in_=dram["w_q"])
    nc.scalar.dma_start(out=w_k, in_=dram["w_k"])
    nc.scalar.dma_start(out=w_v, in_=dram["w_v"])
    nc.scalar.dma_start(out=w_oab, in_=dram["wo_ab"])
    nc.scalar.dma_start(out=w_oc, in_=dram["wo_c"])
    nc.scalar.dma_start(out=bq_d, in_=dram["bq_d"])
    nc.scalar.dma_start(out=bk_d, in_=dram["bk_d"])
    nc.scalar.dma_start(out=bv_bc, in_=dram["bv_bc"])

    xd = dram["xt"]

    def body():
        # ---- input DMAs (sync queue), 512-col chunks, all 6 d-tiles ----
        for ch in range(4):
            cs = slice(ch * 512, (ch + 1) * 512)
            for dt in range(NKT):
                nc.sync.dma_start(out=xt[:, dt, cs], in_=xd[:, dt, cs])

        nm = iter(range(100000))

        # ---- projections (all col-paired), emitted in 2 parts so weave
        # items stay near pair granularity (~0.5us each) ----
        proj_pp = {}

        def proj_q(h, cp, part):
            if part == 0:
                proj_pp[("q", h, cp)] = ppw.tile(
                    [128, 512], f32, name=f"pq{next(nm)}", tag="w")
            pp = proj_pp[("q", h, cp)]
            c0 = slice(cp * 1024, cp * 1024 + 512)
            c1 = slice(cp * 1024 + 512, cp * 1024 + 1024)
            hs = slice(h * 64, h * 64 + 64)
            for dt in range(3 * part, 3 * part + 3):
                nc.tensor.matmul(pp[0:64, :], lhsT=w_q[:, dt, hs],
                                 rhs=xt[:, dt, c0], start=(dt == 0),
                                 stop=(dt == NKT - 1), skip_group_check=True)
                nc.tensor.matmul(pp[64:128, :], lhsT=w_q[:, dt, hs],
                                 rhs=xt[:, dt, c1], start=(dt == 0),
                                 stop=(dt == NKT - 1), skip_group_check=True)
            if part == 1:
                del proj_pp[("q", h, cp)]
                nc.vector.tensor_scalar_add(
                    qt[h][:, cp * 512:(cp + 1) * 512], pp, bq_d[:, h:h + 1])

        def proj_k(h, cp, part, dq=None):
            dq = dq or nc.sync
            if part == 0:
                proj_pp[("k", h, cp)] = ppw.tile(
                    [128, 512], f32, name=f"pk{next(nm)}", tag="w")
            pp = proj_pp[("k", h, cp)]
            c0 = slice(cp * 1024, cp * 1024 + 512)
            c1 = slice(cp * 1024 + 512, cp * 1024 + 1024)
            hs = slice(h * 64, h * 64 + 64)
            for dt in range(3 * part, 3 * part + 3):
                nc.tensor.matmul(pp[0:64, :], lhsT=w_k[:, dt, hs],
                                 rhs=xt[:, dt, c0], start=(dt == 0),
                                 stop=(dt == NKT - 1), skip_group_check=True)
                nc.tensor.matmul(pp[64:128, :], lhsT=w_k[:, dt, hs],
                                 rhs=xt[:, dt, c1], start=(dt == 0),
                                 stop=(dt == NKT - 1), skip_group_check=True)
            if part == 1:
                del proj_pp[("k", h, cp)]
                nc.vector.tensor_scalar_add(kt[h][0:64, c0], pp[0:64, :],
                                            bk_d[0:64, h:h + 1])
                nc.vector.tensor_scalar_add(kt[h][64:128, c1], pp[64:128, :],
                                            bk_d[64:128, h:h + 1])
                dq.dma_start(out=kt[h][64:128, c0], in_=kt[h][0:64, c0])
                dq.dma_start(out=kt[h][0:64, c1], in_=kt[h][64:128, c1])

        def proj_v(sti, part=None):
            """V for all heads, seq chunk sti: col-paired by seq half."""
            a = slice(sti * 128, sti * 128 + 64)
            b = slice(sti * 128 + 64, sti * 128 + 128)
            if part in (0, None):
                proj_pp[("v", sti)] = ppw.tile(
                    [128, 192], f32, name=f"pv{next(nm)}", tag="w")
            pv = proj_pp[("v", sti)]
            dts = range(NKT) if part is None else range(3 * part, 3 * part + 3)
            for dt in dts:
                nc.tensor.matmul(pv[0:64, :], lhsT=xt[:, dt, a],
                                 rhs=w_v[:, dt, :], start=(dt == 0),
                                 stop=(dt == NKT - 1), skip_group_check=True)
                nc.tensor.matmul(pv[64:128, :], lhsT=xt[:, dt, b],
                                 rhs=w_v[:, dt, :], start=(dt == 0),
                                 stop=(dt == NKT - 1), skip_group_check=True)
            if part in (1, None):
                del proj_pp[("v", sti)]
                nc.vector.tensor_tensor(
                    out=v_sb[:, sti, :, :],
                    in0=pv.rearrange("p (h d) -> p h d", h=HLOC),
                    in1=bv_bc.rearrange("p (h d) -> p h d", h=HLOC),
                    op=add)

        # ---- attention pieces ----
        p_tiles = {}
        acc_tiles = {}

        def scores_exp(h, kti, qh):
            if phase < 2:
                return
            ks = slice(kti * 128, (kti + 1) * 128)
            qs = slice(qh * 512, qh * 512 + 512)
            ps = pps.tile([128, 1024], f32, name=f"ps{next(nm)}", tag="ps")
            nc.tensor.matmul(ps[:, 0:512], lhsT=kt[h][0:64, ks],
                             rhs=qt[h][0:64, qs], start=True, stop=True)
            nc.tensor.matmul(ps[:, 512:1024], lhsT=kt[h][64:128, ks],
                             rhs=qt[h][64:128, qs], start=True, stop=True)
            p_t = ppool.tile([128, 1024], bf16, name=f"p{next(nm)}", tag="p")
            nc.scalar.activation(out=p_t, in_=ps, func=Exp, scale=0.125)
            p_tiles[(h, qh, kti)] = p_t

        def attn_v(h, qh, kti, c_only=None):
            """attn@V col-pair + denominator col-pair for one k-tile.
            c_only=0: emit only the c0 half (last pass; c1 runs in tail)."""
            if phase < 3:
                return
            if kti == 0:
                acc_tiles[(h, qh)] = (
                    ppo.tile([128, 512], f32, name=f"po{next(nm)}", tag="po"),
                    ppd.tile([96, 512], f32, name=f"pd{next(nm)}", tag="pd"))
            po, pd = acc_tiles[(h, qh)]
            p_t = p_tiles[(h, qh, kti)]
            st = (kti == 0)
            sp = (kti == 15)
            nc.tensor.matmul(po[0:64, :], lhsT=v_sb[:, kti, h, :],
                             rhs=p_t[:, 0:512], start=st, stop=sp,
                             skip_group_check=True)
            if c_only is None:
                nc.tensor.matmul(po[64:128, :], lhsT=v_sb[:, kti, h, :],
                                 rhs=p_t[:, 512:1024], start=st, stop=sp,
                                 skip_group_check=True)
            if phase >= 4:
                nc.tensor.matmul(pd[0:32, :], lhsT=ones_c,
                                 rhs=p_t[:, 0:512], start=st, stop=sp,
                                 skip_group_check=True)
            if c_only is None:
                if phase >= 4:
                    nc.tensor.matmul(pd[64:96, :], lhsT=ones_c,
                                     rhs=p_t[:, 512:1024], start=st, stop=sp,
                                     skip_group_check=True)
                del p_tiles[(h, qh, kti)]

        def attn_v_c1_tail(h, qh, kti):
            if phase < 3:
                return
            po, pd = acc_tiles[(h, qh)]
            p_t = p_tiles.pop((h, qh, kti))
            st = (kti == 0)
            sp = (kti == 15)
            nc.tensor.matmul(po[64:128, :], lhsT=v_sb[:, kti, h, :],
                             rhs=p_t[:, 512:1024], start=st, stop=sp,
                             skip_group_check=True)
            nc.tensor.matmul(pd[64:96, :], lhsT=ones_c,
                             rhs=p_t[:, 512:1024], start=st, stop=sp,
                             skip_group_check=True)

        def norm_c(h, qh, c):
            """Normalize one c-chunk of pass (h, qh) into attnT.

            Cross-partition DVE ops cost ~2us on HW, so the multiply is
            always partition-aligned with po; when the result's home half
            differs, a cheap sbuf->sbuf DMA (gpsimd queue) moves it."""
            if phase < 4:
                return
            po, pd = acc_tiles[(h, qh)]
            if c == 1:
                acc_tiles.pop((h, qh))
            bs = slice(64 * c, 64 * c + 64)
            r_t = rpool.tile([1, 512], f32, name=f"r{next(nm)}", tag="r")
            # gpsimd partition_broadcast mis-addresses at partition offset
            # 64 (HW-verified): keep r at base 0, broadcast all 128 rows.
            nc.vector.reciprocal(out=r_t, in_=pd[64 * c:64 * c + 1, :])
            b_t = bpool.tile([128, 512], f32, name=f"b{next(nm)}", tag="b")
            nc.gpsimd.partition_broadcast(b_t, r_t)
            qs = slice(qh * 1024 + c * 512, qh * 1024 + (c + 1) * 512)
            src = po[bs, :]
            bb = b_t[bs, :]
            home = 0 if h == 2 else h          # attnT half the head lives in
            dst_t = attnTc if h == 2 else attnT
            if c == home:
                nc.vector.tensor_tensor(out=dst_t[64 * home:64 * home + 64, qs],
                                        in0=src, in1=bb, op=mult)
            else:
                s_t = spool.tile([128, 512], bf16, name=f"sg{next(nm)}",
                                 tag="sg")
                nc.vector.tensor_tensor(out=s_t[bs, :], in0=src, in1=bb,
                                        op=mult)
                nc.sync.dma_start(
                    out=dst_t[64 * home:64 * home + 64, qs], in_=s_t[bs, :])

        def norm(h, qh):
            norm_c(h, qh, 0)
            norm_c(h, qh, 1)

        def wo_echunk(e, qh):
            """Out-proj, w_o stationary: out^T[e*128:(e+1)*128, qh half].
            Stationary weights are reused across the two q-chunks, so
            LDWEIGHTS amortizes; A+B ride one K=128 matmul, C chains."""
            if phase < 5:
                return
            esl = slice(e * 128, (e + 1) * 128)
            o_t = opool.tile([128, 1024], f32, name=f"o{next(nm)}", tag="o")
            for qc in range(2):
                qsl = slice(qh * 1024 + qc * 512, qh * 1024 + (qc + 1) * 512)
                pw = ppw.tile([128, 512], f32, name=f"pwo{next(nm)}", tag="w")
                nc.tensor.matmul(pw, lhsT=w_oab[:, esl], rhs=attnT[:, qsl],
                                 start=True, stop=False)
                nc.tensor.matmul(pw, lhsT=w_oc[0:64, esl],
                                 rhs=attnTc[0:64, qsl],
                                 start=False, stop=True)
                nc.vector.tensor_copy(out=o_t[:, qc * 512:(qc + 1) * 512],
                                      in_=pw)
            nc.sync.dma_start(
                out=dram["out"][esl, qh * 1024:(qh + 1) * 1024], in_=o_t)

        # ---- pass schedule ----
        # passes: (qh, h); last pass gets the split-c tail treatment.
        passes = [(0, 0), (0, 1), (0, 2), (1, 2), (1, 0), (1, 1)]

        # Prologue: minimal work to issue the first exp, then weave.
        proj_k(0, 0, 0, dq=nc.scalar)
        proj_k(0, 0, 1, dq=nc.scalar)
        proj_q(0, 0, 0)
        proj_q(0, 0, 1)
        proj_v(0)

        # Per-slot weave of ~0.5us items. Readiness rules:
        #  - pass 0 runs attn@V at lag 3: proj_v(j) complete by slot j+2
        #  - proj parts: part0/part1 in adjacent slots (shared ppw tile)
        #  - norm_c(p) at slots 0/1 of pass p+1 (frees po/pd)
        #  - wo for qh0 only after norm(2,0); for qh1 in the tail
        def W(pairs):
            out = [[] for _ in range(16)]
            for slot, fn in pairs:
                out[slot].append(fn)
            return out

        def vparts(j):
            return [(max(0, j - 2), lambda j=j: proj_v(j, 0)),
                    (max(1, j - 1), lambda j=j: proj_v(j, 1))]

        weave = {
            0: W(sum((vparts(j) for j in range(1, 16)), [])
                 + [(4, lambda: proj_k(0, 1, 0)), (5, lambda: proj_k(0, 1, 1)),
                    (10, lambda: proj_k(1, 0, 0)),
                    (11, lambda: proj_k(1, 0, 1)),
                    (13, lambda: proj_q(1, 0, 0)),
                    (14, lambda: proj_q(1, 0, 1))]),
            1: W([(0, lambda: norm_c(0, 0, 0)), (1, lambda: norm_c(0, 0, 1)),
                  (2, lambda: proj_k(1, 1, 0)), (3, lambda: proj_k(1, 1, 1)),
                  (6, lambda: proj_k(2, 0, 0)), (7, lambda: proj_k(2, 0, 1)),
                  (9, lambda: proj_k(2, 1, 0)), (10, lambda: proj_k(2, 1, 1)),
                  (12, lambda: proj_q(2, 0, 0)),
                  (13, lambda: proj_q(2, 0, 1))]),
            2: W([(0, lambda: norm_c(1, 0, 0)), (1, lambda: norm_c(1, 0, 1)),
                  (3, lambda: proj_q(2, 1, 0)), (4, lambda: proj_q(2, 1, 1)),
                  (8, lambda: proj_q(0, 1, 0)), (9, lambda: proj_q(0, 1, 1))]),
            3: W([(0, lambda: norm_c(2, 0, 0)), (1, lambda: norm_c(2, 0, 1)),
                  (2, lambda: proj_q(1, 1, 0)), (3, lambda: proj_q(1, 1, 1)),
                  (5, lambda: wo_echunk(0, 0)), (8, lambda: wo_echunk(1, 0)),
                  (11, lambda: wo_echunk(2, 0)),
                  (14, lambda: wo_echunk(3, 0))]),
            4: W([(0, lambda: norm_c(2, 1, 0)), (1, lambda: norm_c(2, 1, 1)),
                  (3, lambda: wo_echunk(4, 0)), (7, lambda: wo_echunk(5, 0))]),
            5: W([(0, lambda: norm_c(0, 1, 0)), (1, lambda: norm_c(0, 1, 1))]),
        }
        lags = [3, 2, 2, 2, 2, 2]

        for pi, (qh, h) in enumerate(passes):
            lag = lags[pi]
            for kti in range(16):
                scores_exp(h, kti, qh)
                if kti >= lag:
                    attn_v(h, qh, kti - lag)
                for fn in weave[pi][kti]:
                    fn()
            for k in range(16 - lag, 16):
                attn_v(h, qh, k)

        # ---- tail: norm the last pass, then Wo(qh1) ----
        norm_c(1, 1, 0)
        norm_c(1, 1, 1)
        for e in range(6):
            wo_echunk(e, 1)

    if loop_n is None:
        body()
    else:
        with tc.For_i(0, loop_n, 1):
            body()


def _build(loop_n=None, phase=5):
    from contextlib import ExitStack

    import concourse.bacc as bacc
    import concourse.mybir as mybir
    import concourse.tile as tile

    f32 = mybir.dt.float32
    bf16 = mybir.dt.bfloat16
    nc = bacc.Bacc("TRN2", target_bir_lowering=False, debug=False,
                   num_devices=8)
    dram = {
        "xt": nc.dram_tensor("xt", [128, NKT, S], bf16,
                             kind="ExternalInput").ap(),
        "w_q": nc.dram_tensor("w_q", [128, NKT, 192], bf16,
                              kind="ExternalInput").ap(),
        "w_k": nc.dram_tensor("w_k", [128, NKT, 192], bf16,
                              kind="ExternalInput").ap(),
        "w_v": nc.dram_tensor("w_v", [128, NKT, 192], bf16,
                              kind="ExternalInput").ap(),
        "wo_ab": nc.dram_tensor("wo_ab", [128, D], bf16,
                                kind="ExternalInput").ap(),
        "wo_c": nc.dram_tensor("wo_c", [64, D], bf16,
                               kind="ExternalInput").ap(),
        "bq_d": nc.dram_tensor("bq_d", [128, HLOC], f32,
                               kind="ExternalInput").ap(),
        "bk_d": nc.dram_tensor("bk_d", [128, HLOC], f32,
                               kind="ExternalInput").ap(),
        "bv_bc": nc.dram_tensor("bv_bc", [128, HLOC * 64], f32,
                                kind="ExternalInput").ap(),
        "out": nc.dram_tensor("out", [D, S], f32, kind="ExternalOutput").ap(),
    }
    with tile.TileContext(nc) as tc:
        with ExitStack() as ctx:
            _emit(nc, tc, ctx, dram, loop_n=loop_n, phase=phase)
    nc.compile()
    return nc


def _get_nc():
    if "nc" not in _CACHE:
        _CACHE["nc"] = _build()
    return _CACHE["nc"]


def _shard(inputs):
    import ml_dtypes

    bf = ml_dtypes.bfloat16
    x = np.asarray(inputs["x"], np.float32)
    Wq = np.asarray(inputs["Wq"], np.float32)
    Wk = np.asarray(inputs["Wk"], np.float32)
    Wv = np.asarray(inputs["Wv"], np.float32)
    Wo = np.asarray(inputs["Wo"], np.float32)
    bq = np.asarray(inputs["bq"], np.float32)
    bk = np.asarray(inputs["bk"], np.float32)
    bv = np.asarray(inputs["bv"], np.float32)

    def wtiles(w):  # [768, C] -> [128, 6, C]
        return np.ascontiguousarray(
            w.reshape(NKT, 128, -1).transpose(1, 0, 2)).astype(bf)

    def bias_dup(b):  # [192] -> [128, 3] halves-duplicated per head
        out = np.zeros((128, HLOC), np.float32)
        for h in range(HLOC):
            out[0:64, h] = b[h * 64:(h + 1) * 64]
            out[64:128, h] = b[h * 64:(h + 1) * 64]
        return out

    xts = []
    for b in range(2):
        xts.append(np.ascontiguousarray(
            x[b].T.reshape(NKT, 128, S).transpose(1, 0, 2)).astype(bf))

    in_maps = []
    for c in range(8):
        b, g = divmod(c, 4)
        o = 192 * g
        in_maps.append({
            "xt": xts[b],
            "w_q": wtiles(Wq[:, o:o + 192]),
            "w_k": wtiles(Wk[:, o:o + 192]),
            "w_v": wtiles(Wv[:, o:o + 192]),
            "wo_ab": np.ascontiguousarray(Wo[o:o + 128, :]).astype(bf),
            "wo_c": np.ascontiguousarray(Wo[o + 128:o + 192, :]).astype(bf),
            "bq_d": bias_dup(bq[o:o + 192]),
            "bk_d": bias_dup(bk[o:o + 192]),
            "bv_bc": np.ascontiguousarray(
                np.broadcast_to(bv[o:o + 192], (128, 192))),
        })
    return in_maps


def kernel(x, Wq, bq, Wk, bk, Wv, bv, Wo, bo):
    from concourse.bass_utils import run_bass_kernel_spmd

    nc = _get_nc()
    in_maps = _shard(dict(x=x, Wq=Wq, Wk=Wk, Wv=Wv, Wo=Wo,
                          bq=bq, bk=bk, bv=bv))
    res = run_bass_kernel_spmd(nc, in_maps, core_ids=list(range(8)))
    out = np.zeros((2, S, D), np.float32)
    for c in range(8):
        out[c // 4] += res.results[c]["out"].T
    out += np.asarray(bo, np.float32)
    return out
